# revision 6
# baseline (speedup 1.0000x reference)
"""HBitLinear Trainium2 kernel (v2).

out = quant4(x @ H_1024) @ ternary(W).T, x:[8,8192,1024] f32, W:[1024,1024] f32.

Strategy (8 NeuronCores, data-parallel over the batch dim):
  - Each core gets one batch slice x_b [8192,1024]; W is replicated.
  - Hadamard via Kronecker split H_1024 = H2 (x) H2 (x) H2 (x) H_128:
    two H2 butterfly stages on DVE/GpSimd, and the LAST H2 stage is folded
    into the H128 matmul by using a doubled moving operand [hm|hm] /
    [hm|-hm] at N=256, which unlocks the float32r fast path (1 cyc/row vs
    4 for fp32) on the PE.
  - Quantization: per-token absmax -> rsc; ONE scalar pass computes
    t = xh*rsc + MAGIC (fp32 RNE trick), t is transposed on the PE in fp32
    (2 cyc/row), and the -MAGIC + fp8 cast rides the mandatory PSUM->SBUF
    copy as a second scalar pass.  This saves a whole scalar pass and the
    bf16 transpose round-trip of v1.
  - M2 runs fp8 DoubleRow with the kk-loop outermost so each weight load
    serves both 512-wide output halves.
  - Epilogue: out = G * scale[token] * wscale[out_feature] on DVE.
"""

import numpy as np

_CACHE: dict = {}

P = 128          # partitions
ST = 64          # token tiles per core (8192 / 128)
NCHUNK = 8       # 1024 / 128
MAGIC = float(np.float32(3 * 2 ** 22))  # 1.5*2^23: fp32 RNE rounding constant


def _sylvester(k: int) -> np.ndarray:
    h = np.array([[1]], dtype=np.int64)
    for _ in range(k):
        h = np.block([[h, h], [h, -h]])
    return h


def _build():
    import concourse.bass as bass  # noqa: F401
    import concourse.mybir as mybir
    import concourse.tile as tile
    from concourse import bacc

    dt = mybir.dt
    ALU = mybir.AluOpType
    ACTF = mybir.ActivationFunctionType

    nc = bacc.Bacc("TRN2", target_bir_lowering=False, debug=False)

    x = nc.dram_tensor("x", [ST * P, NCHUNK * P], dt.float32, kind="ExternalInput")
    w = nc.dram_tensor("w", [NCHUNK * P, NCHUNK * P], dt.float32, kind="ExternalInput")
    # hmx[:, 0:2, :] = [hm, hm], hmx[:, 2:4, :] = [hm, -hm]  (hm = H128/32)
    hmx = nc.dram_tensor("hmx", [P, 4, P], dt.float32r, kind="ExternalInput")
    out = nc.dram_tensor("out", [ST * P, NCHUNK * P], dt.float32, kind="ExternalOutput")

    from contextlib import ExitStack

    f32r = dt.float32r

    with tile.TileContext(nc) as tc, ExitStack() as stack:
        # ---------------- persistent constants ----------------
        const = stack.enter_context(tc.tile_pool(name="const", bufs=1))
        hmx_sb = const.tile([P, 4, P], dt.float32r, tag="hmx")
        nc.sync.dma_start(hmx_sb[:], hmx[:])
        id32 = const.tile([P, P], dt.float32, tag="id32")
        id8 = const.tile([P, P], dt.float8e4, tag="id8")
        from concourse.masks import make_identity
        make_identity(nc, id32[:])
        make_identity(nc, id8[:])
        # ternary weight, transposed: ternT[jc] [j2=128, o=1024] fp8
        ternT = const.tile([P, NCHUNK, P * NCHUNK], dt.float8e4, tag="ternT")
        # broadcast weight scales [128, 1024] fp32
        wsb = const.tile([P, P * NCHUNK], dt.float32, tag="wsb")

        # Main-loop PSUM pools: exactly 8 banks.
        # ps_a holds xT (slot 0) and tT (slot 1) per tile via same-tag
        # rotation: 2 slots x 4KB = 4 banks.  ps_xh 2 banks, ps_g 2 banks.
        ps_a = stack.enter_context(tc.tile_pool(name="ps_a", bufs=2, space="PSUM"))
        ps_xh = stack.enter_context(tc.tile_pool(name="ps_xh", bufs=1, space="PSUM"))
        ps_g = stack.enter_context(tc.tile_pool(name="ps_g", bufs=1, space="PSUM"))

        # ---------------- weight prep (one-time) ----------------
        ws_dram = nc.dram_tensor("ws_scratch", [NCHUNK * P], dt.float32)
        with tc.tile_pool(name="wprep", bufs=1) as wp:
            w_sb = wp.tile([P, NCHUNK, P * NCHUNK], dt.float32, tag="w")
            nc.sync.dma_start(
                w_sb[:],
                w[:].rearrange("(a p) j -> p a j", p=P),
            )
            ws = wp.tile([P, NCHUNK], dt.float32, tag="ws")
            bpos = wp.tile([P, NCHUNK], dt.float32, tag="bpos")
            bneg = wp.tile([P, NCHUNK], dt.float32, tag="bneg")
            tlt = wp.tile([P, NCHUNK, P * NCHUNK], dt.float32, tag="tlt")
            tern = wp.tile([P, NCHUNK, P * NCHUNK], dt.float8e4, tag="tern")
            for oc in range(NCHUNK):
                # ws = max(mean|w|, 1e-5) per row
                nc.vector.tensor_reduce(
                    ws[:, oc : oc + 1], w_sb[:, oc, :],
                    axis=mybir.AxisListType.X, op=ALU.add,
                    apply_absolute_value=True,
                )
                nc.vector.tensor_scalar(
                    ws[:, oc : oc + 1], ws[:, oc : oc + 1],
                    float(np.float32(1.0 / 1024.0)), 1e-5, ALU.mult, ALU.max,
                )
                nc.vector.tensor_scalar_mul(bpos[:, oc : oc + 1], ws[:, oc : oc + 1], 0.5)
                nc.vector.tensor_scalar_mul(bneg[:, oc : oc + 1], ws[:, oc : oc + 1], -0.5)
                # tern = (w > 0.5 ws) - (w < -0.5 ws)  in {-1, 0, 1}
                nc.vector.tensor_scalar(
                    tlt[:, oc, :], w_sb[:, oc, :],
                    bneg[:, oc : oc + 1], None, ALU.is_lt,
                )
                nc.vector.scalar_tensor_tensor(
                    tern[:, oc, :], w_sb[:, oc, :], bpos[:, oc : oc + 1],
                    tlt[:, oc, :], ALU.is_gt, ALU.subtract,
                )
            # transpose tern blocks -> ternT, borrowing the ps_g slot
            for jc in range(NCHUNK):
                tp = ps_g.tile([P, NCHUNK, P, 2], dt.float8e4, tag="g")
                for oc in range(NCHUNK):
                    nc.tensor.transpose(
                        tp[:, oc, :, 0], tern[:, oc, jc * P : (jc + 1) * P], id8[:]
                    )
                nc.scalar.copy(ternT[:, jc, :], tp[:, :, :, 0])
            # wscale broadcast tile: bounce through DRAM, then a partition-
            # stride-0 DMA broadcasts the 1024-vector to all 128 partitions.
            nc.sync.dma_start(
                ws_dram[:].rearrange("(a p) -> p a", p=P), ws[:, :]
            )
            wsb_bcast = bass.AP(
                tensor=ws_dram[:].tensor, offset=0,
                ap=[[0, P]] + list(ws_dram[:].ap),
            )
            nc.gpsimd.dma_start(out=wsb[:, :], in_=wsb_bcast)

        # ---------------- main loop pools ----------------
        xpool = stack.enter_context(tc.tile_pool(name="xin", bufs=4))
        bfly = stack.enter_context(tc.tile_pool(name="bfly", bufs=2))
        qpool = stack.enter_context(tc.tile_pool(name="q", bufs=2))
        opool = stack.enter_context(tc.tile_pool(name="osb", bufs=3))
        scpool = stack.enter_context(tc.tile_pool(name="scales", bufs=4))

        for st in range(ST):
            s0 = st * P
            x_t = xpool.tile([P, NCHUNK * P], dt.float32, tag="x")
            nc.sync.dma_start(x_t[:], x[s0 : s0 + P, :])

            # transpose x chunks: xT[(b2,b1,b0)] = x_block(c).T   [i2, s]
            # chunk bits (b2, b1, b0) = (c>>2, c>>1, c) & 1
            xT = ps_a.tile([P, 2, 2, 2, P], dt.float32, tag="ab")
            for c in range(NCHUNK):
                nc.tensor.transpose(
                    xT[:, (c >> 2) & 1, (c >> 1) & 1, c & 1, :],
                    x_t[:, c * P : (c + 1) * P], id32[:],
                )

            # FHT8: stage 1 (mix b2) on DVE, stage 2 (mix b1) on GpSimd,
            # stage 3 (mix b0) folded into the M1 matmul's moving operand.
            v0e = bfly.tile([P, 2, 2, P], dt.float32, tag="v0e")
            nc.scalar.copy(v0e[:, :, :, :], xT[:, 0, :, :, :])
            v1 = bfly.tile([P, 2, 2, 2, P], dt.float32, tag="v1")
            v2 = bfly.tile([P, 2, 2, 2, P], dt.float32r, tag="v2")
            nc.vector.tensor_add(v1[:, 0, :, :, :], v0e[:], xT[:, 1, :, :, :])
            nc.vector.tensor_sub(v1[:, 1, :, :, :], v0e[:], xT[:, 1, :, :, :])
            nc.gpsimd.tensor_add(v2[:, :, 0, :, :], v1[:, :, 0, :, :], v1[:, :, 1, :, :])
            nc.gpsimd.tensor_sub(v2[:, :, 1, :, :], v1[:, :, 0, :, :], v1[:, :, 1, :, :])

            # M1 + folded stage 3: for each (p2,p1) output pair, accumulate
            #   xh[:, (p2 p1 p0), j2] += v2[p2,p1,b0].T @ [hm | (-1)^p0 hm]
            # f32r moving operand at N=256 -> 1 cyc/row.
            xh = ps_xh.tile([P, NCHUNK, P], dt.float32, tag="xh")
            for p2 in range(2):
                for p1 in range(2):
                    k2 = 2 * (2 * p2 + p1)
                    for b0 in range(2):
                        nc.tensor.matmul(
                            xh[:, k2 : k2 + 2, :],
                            v2[:, p2, p1, b0, :],
                            hmx_sb[:, 2 * b0 : 2 * b0 + 2, :],
                            start=(b0 == 0), stop=(b0 == 1),
                        )

            # per-token scale
            amax = scpool.tile([P, 1], dt.float32, tag="amax")
            sc = scpool.tile([P, 1], dt.float32, tag="sc")
            rsc = scpool.tile([P, 1], dt.float32, tag="rsc")
            nc.vector.tensor_reduce(
                amax[:], xh[:, :, :], axis=mybir.AxisListType.XY, op=ALU.max,
                apply_absolute_value=True,
            )
            nc.vector.tensor_scalar(
                sc[:], amax[:], 1e-5, float(np.float32(1.0 / 7.0)), ALU.max, ALU.mult
            )
            nc.vector.reciprocal(rsc[:], sc[:])

            # quantize pass 1: t = xh * rsc + MAGIC  (fp32, RNE in low bits)
            t_t = qpool.tile([P, NCHUNK, P], dt.float32, tag="t")
            nc.scalar.activation(t_t[:, :, :], xh[:, :, :], ACTF.Copy,
                                 bias=MAGIC, scale=rsc[:])

            # transpose t chunks in fp32 (2 cyc/row), then the PSUM->SBUF
            # copy subtracts MAGIC and casts to fp8 in one scalar pass.
            tT = ps_a.tile([P, NCHUNK, P], dt.float32, tag="ab")
            for c in range(NCHUNK):
                nc.tensor.transpose(tT[:, c, :], t_t[:, c, :], id32[:])
            qT = qpool.tile([P, NCHUNK, P], dt.float8e4, tag="qT")
            nc.scalar.activation(qT[:, :, :], tT[:, :, :], ACTF.Copy, bias=-MAGIC)

            # M2: G = q @ tern.T (fp8 DoubleRow, exact); kk outer so each
            # weight load serves both 512-wide output halves.
            g = ps_g.tile([P, 2, 512], dt.float32, tag="g")
            for kk in range(NCHUNK // 2):
                for oh in range(2):
                    nc.tensor.matmul(
                        g[:, oh, :], qT[:, 2 * kk : 2 * kk + 2, :],
                        ternT[:, 2 * kk : 2 * kk + 2, oh * 512 : (oh + 1) * 512],
                        start=(kk == 0), stop=(kk == NCHUNK // 2 - 1),
                        perf_mode=mybir.MatmulPerfMode.DoubleRow,
                        skip_group_check=True,
                    )

            # epilogue: out = G * sc[token] * wscale[out_feature]
            o_t = opool.tile([P, NCHUNK * P], dt.float32, tag="o")
            for oh in range(2):
                nc.vector.scalar_tensor_tensor(
                    o_t[:, oh * 512 : (oh + 1) * 512], g[:, oh, :], sc[:],
                    wsb[:, oh * 512 : (oh + 1) * 512], ALU.mult, ALU.mult,
                )
            nc.sync.dma_start(out[s0 : s0 + P, :], o_t[:])

    nc.finalize()
    return nc


def _get_nc():
    if "nc" not in _CACHE:
        _CACHE["nc"] = _build()
    return _CACHE["nc"]


def _make_hmx() -> np.ndarray:
    hm = (_sylvester(7).astype(np.float32) / np.float32(32.0)).astype(np.float32)
    return np.ascontiguousarray(
        np.stack([hm, hm, hm, -hm], axis=1)
    )  # [128, 4, 128]


def _in_maps(x: np.ndarray, weight: np.ndarray) -> list:
    hmx = _make_hmx()
    w32 = np.ascontiguousarray(weight, dtype=np.float32)
    return [
        {"x": np.ascontiguousarray(x[i]), "w": w32, "hmx": hmx} for i in range(8)
    ]


def kernel(x: np.ndarray, weight: np.ndarray) -> np.ndarray:
    from concourse.bass_utils import run_bass_kernel_spmd

    assert x.shape == (8, ST * P, NCHUNK * P) and x.dtype == np.float32
    assert weight.shape == (NCHUNK * P, NCHUNK * P)

    nc = _get_nc()
    res = run_bass_kernel_spmd(nc, _in_maps(x, weight), core_ids=list(range(8)))
    return np.stack([res.results[i]["out"] for i in range(8)], axis=0)


# revision 8
# speedup vs baseline: 2.1087x; 2.1087x over previous
"""HBitLinear Trainium2 kernel (v2).

out = quant4(x @ H_1024) @ ternary(W).T, x:[8,8192,1024] f32, W:[1024,1024] f32.

Strategy (8 NeuronCores, data-parallel over the batch dim):
  - Each core gets one batch slice x_b [8192,1024]; W is replicated.
  - Hadamard via Kronecker split H_1024 = H2 (x) H2 (x) H2 (x) H_128:
    two H2 butterfly stages on DVE/GpSimd, and the LAST H2 stage is folded
    into the H128 matmul by using a doubled moving operand [hm|hm] /
    [hm|-hm] at N=256, which unlocks the float32r fast path (1 cyc/row vs
    4 for fp32) on the PE.
  - Quantization: per-token absmax -> rsc; ONE scalar pass computes
    t = xh*rsc + MAGIC (fp32 RNE trick), t is transposed on the PE in fp32
    (2 cyc/row), and the -MAGIC + fp8 cast rides the mandatory PSUM->SBUF
    copy as a second scalar pass.  This saves a whole scalar pass and the
    bf16 transpose round-trip of v1.
  - M2 runs fp8 DoubleRow with the kk-loop outermost so each weight load
    serves both 512-wide output halves.
  - Epilogue: out = G * scale[token] * wscale[out_feature] on DVE.
"""

import numpy as np

_CACHE: dict = {}

P = 128          # partitions
ST = 64          # token tiles per core (8192 / 128)
NCHUNK = 8       # 1024 / 128
MAGIC = float(np.float32(3 * 2 ** 22))  # 1.5*2^23: fp32 RNE rounding constant


def _sylvester(k: int) -> np.ndarray:
    h = np.array([[1]], dtype=np.int64)
    for _ in range(k):
        h = np.block([[h, h], [h, -h]])
    return h


def _build():
    import concourse.bass as bass  # noqa: F401
    import concourse.mybir as mybir
    import concourse.tile as tile
    from concourse import bacc

    dt = mybir.dt
    ALU = mybir.AluOpType
    ACTF = mybir.ActivationFunctionType

    nc = bacc.Bacc("TRN2", target_bir_lowering=False, debug=False)

    x = nc.dram_tensor("x", [ST * P, NCHUNK * P], dt.float32, kind="ExternalInput")
    w = nc.dram_tensor("w", [NCHUNK * P, NCHUNK * P], dt.float32, kind="ExternalInput")
    # hmx[:, 0:2, :] = [hm, hm], hmx[:, 2:4, :] = [hm, -hm]  (hm = H128/32)
    hmx = nc.dram_tensor("hmx", [P, 4, P], dt.float32r, kind="ExternalInput")
    out = nc.dram_tensor("out", [ST * P, NCHUNK * P], dt.float32, kind="ExternalOutput")

    from contextlib import ExitStack

    f32r = dt.float32r

    with tile.TileContext(nc) as tc, ExitStack() as stack:
        # ---------------- persistent constants ----------------
        const = stack.enter_context(tc.tile_pool(name="const", bufs=1))
        hmx_sb = const.tile([P, 4, P], dt.float32r, tag="hmx")
        nc.sync.dma_start(hmx_sb[:], hmx[:])
        id32 = const.tile([P, P], dt.float32, tag="id32")
        id8 = const.tile([P, P], dt.float8e4, tag="id8")
        from concourse.masks import make_identity
        make_identity(nc, id32[:])
        make_identity(nc, id8[:])
        # ternary weight, transposed: ternT[jc] [j2=128, o=1024] fp8
        ternT = const.tile([P, NCHUNK, P * NCHUNK], dt.float8e4, tag="ternT")
        # broadcast weight scales [128, 1024] fp32
        wsb = const.tile([P, P * NCHUNK], dt.float32, tag="wsb")

        # Main-loop PSUM pools: exactly 8 banks, all single-buffered; the
        # software pipeline below provides the cross-tile overlap instead.
        ps_xT = stack.enter_context(tc.tile_pool(name="ps_xT", bufs=1, space="PSUM"))
        ps_xh = stack.enter_context(tc.tile_pool(name="ps_xh", bufs=1, space="PSUM"))
        ps_tT = stack.enter_context(tc.tile_pool(name="ps_tT", bufs=1, space="PSUM"))
        ps_g = stack.enter_context(tc.tile_pool(name="ps_g", bufs=1, space="PSUM"))

        # ---------------- weight prep (one-time) ----------------
        ws_dram = nc.dram_tensor("ws_scratch", [NCHUNK * P], dt.float32)
        with tc.tile_pool(name="wprep", bufs=1) as wp:
            w_sb = wp.tile([P, NCHUNK, P * NCHUNK], dt.float32, tag="w")
            nc.sync.dma_start(
                w_sb[:],
                w[:].rearrange("(a p) j -> p a j", p=P),
            )
            ws = wp.tile([P, NCHUNK], dt.float32, tag="ws")
            bpos = wp.tile([P, NCHUNK], dt.float32, tag="bpos")
            bneg = wp.tile([P, NCHUNK], dt.float32, tag="bneg")
            tlt = wp.tile([P, NCHUNK, P * NCHUNK], dt.float32, tag="tlt")
            tern = wp.tile([P, NCHUNK, P * NCHUNK], dt.float8e4, tag="tern")
            for oc in range(NCHUNK):
                # ws = max(mean|w|, 1e-5) per row
                nc.vector.tensor_reduce(
                    ws[:, oc : oc + 1], w_sb[:, oc, :],
                    axis=mybir.AxisListType.X, op=ALU.add,
                    apply_absolute_value=True,
                )
                nc.vector.tensor_scalar(
                    ws[:, oc : oc + 1], ws[:, oc : oc + 1],
                    float(np.float32(1.0 / 1024.0)), 1e-5, ALU.mult, ALU.max,
                )
                nc.vector.tensor_scalar_mul(bpos[:, oc : oc + 1], ws[:, oc : oc + 1], 0.5)
                nc.vector.tensor_scalar_mul(bneg[:, oc : oc + 1], ws[:, oc : oc + 1], -0.5)
                # tern = (w > 0.5 ws) - (w < -0.5 ws)  in {-1, 0, 1}
                nc.vector.tensor_scalar(
                    tlt[:, oc, :], w_sb[:, oc, :],
                    bneg[:, oc : oc + 1], None, ALU.is_lt,
                )
                nc.vector.scalar_tensor_tensor(
                    tern[:, oc, :], w_sb[:, oc, :], bpos[:, oc : oc + 1],
                    tlt[:, oc, :], ALU.is_gt, ALU.subtract,
                )
            # transpose tern blocks -> ternT, borrowing the ps_g slot
            for jc in range(NCHUNK):
                tp = ps_g.tile([P, NCHUNK, P, 2], dt.float8e4, tag="g")
                for oc in range(NCHUNK):
                    nc.tensor.transpose(
                        tp[:, oc, :, 0], tern[:, oc, jc * P : (jc + 1) * P], id8[:]
                    )
                nc.scalar.copy(ternT[:, jc, :], tp[:, :, :, 0])
            # wscale broadcast tile: bounce through DRAM, then a partition-
            # stride-0 DMA broadcasts the 1024-vector to all 128 partitions.
            nc.sync.dma_start(
                ws_dram[:].rearrange("(a p) -> p a", p=P), ws[:, :]
            )
            wsb_bcast = bass.AP(
                tensor=ws_dram[:].tensor, offset=0,
                ap=[[0, P]] + list(ws_dram[:].ap),
            )
            nc.gpsimd.dma_start(out=wsb[:, :], in_=wsb_bcast)

        # ---------------- main loop pools ----------------
        xpool = stack.enter_context(tc.tile_pool(name="xin", bufs=4))
        bfly = stack.enter_context(tc.tile_pool(name="bfly", bufs=2))
        qpool = stack.enter_context(tc.tile_pool(name="q", bufs=2))
        opool = stack.enter_context(tc.tile_pool(name="osb", bufs=3))
        scpool = stack.enter_context(tc.tile_pool(name="scales", bufs=8))

        # Per-tile live state for the software pipeline.
        S: list[dict] = [dict() for _ in range(ST)]

        def phase_load(t):
            s = S[t]
            s["x"] = xpool.tile([P, NCHUNK * P], dt.float32, tag="x", name=f"x{t}")
            nc.sync.dma_start(s["x"][:], x[t * P : (t + 1) * P, :])

        def phase_a(t):
            # inX transposes + v0e copy + stage1 (DVE) + stage2 (GpSimd)
            s = S[t]
            xT = ps_xT.tile([P, 2, 2, 2, P], dt.float32, tag="xT", name=f"xT{t}")
            x_t = s["x"]
            for c in range(NCHUNK):
                nc.tensor.transpose(
                    xT[:, (c >> 2) & 1, (c >> 1) & 1, c & 1, :],
                    x_t[:, c * P : (c + 1) * P], id32[:],
                )
            v0e = bfly.tile([P, 2, 2, P], dt.float32, tag="v0e", name=f"v0e{t}")
            nc.scalar.copy(v0e[:, :, :, :], xT[:, 0, :, :, :])
            v1 = bfly.tile([P, 2, 2, 2, P], dt.float32, tag="v1", name=f"v1{t}")
            v2 = bfly.tile([P, 2, 2, 2, P], dt.float32r, tag="v2", name=f"v2{t}")
            nc.vector.tensor_add(v1[:, 0, :, :, :], v0e[:], xT[:, 1, :, :, :])
            nc.vector.tensor_sub(v1[:, 1, :, :, :], v0e[:], xT[:, 1, :, :, :])
            nc.gpsimd.tensor_add(v2[:, :, 0, :, :], v1[:, :, 0, :, :], v1[:, :, 1, :, :])
            nc.gpsimd.tensor_sub(v2[:, :, 1, :, :], v1[:, :, 0, :, :], v1[:, :, 1, :, :])
            s["v2"] = v2

        def phase_b(t):
            # M1 (+ folded stage 3) and the per-token scale chain.
            s = S[t]
            v2 = s["v2"]
            xh = ps_xh.tile([P, NCHUNK, P], dt.float32, tag="xh", name=f"xh{t}")
            for p2 in range(2):
                for p1 in range(2):
                    k2 = 2 * (2 * p2 + p1)
                    for b0 in range(2):
                        nc.tensor.matmul(
                            xh[:, k2 : k2 + 2, :],
                            v2[:, p2, p1, b0, :],
                            hmx_sb[:, 2 * b0 : 2 * b0 + 2, :],
                            start=(b0 == 0), stop=(b0 == 1),
                        )
            amax = scpool.tile([P, 1], dt.float32, tag="amax", name=f"amax{t}")
            sc = scpool.tile([P, 1], dt.float32, tag="sc", name=f"sc{t}")
            rsc = scpool.tile([P, 1], dt.float32, tag="rsc", name=f"rsc{t}")
            nc.vector.tensor_reduce(
                amax[:], xh[:, :, :], axis=mybir.AxisListType.XY, op=ALU.max,
                apply_absolute_value=True,
            )
            nc.vector.tensor_scalar(
                sc[:], amax[:], 1e-5, float(np.float32(1.0 / 7.0)), ALU.max, ALU.mult
            )
            nc.vector.reciprocal(rsc[:], sc[:])
            # quantize pass 1: t = xh * rsc + MAGIC  (fp32, RNE in low bits)
            t_t = qpool.tile([P, NCHUNK, P], dt.float32, tag="t", name=f"t{t}")
            nc.scalar.activation(t_t[:, :, :], xh[:, :, :], ACTF.Copy,
                                 bias=MAGIC, scale=rsc[:])
            s["sc"], s["t"] = sc, t_t

        def phase_c(t):
            # transpose t in fp32; -MAGIC + fp8 cast rides the PSUM->SBUF copy
            s = S[t]
            t_t = s["t"]
            tT = ps_tT.tile([P, NCHUNK, P], dt.float32, tag="tT", name=f"tT{t}")
            for c in range(NCHUNK):
                nc.tensor.transpose(tT[:, c, :], t_t[:, c, :], id32[:])
            qT = qpool.tile([P, NCHUNK, P], dt.float8e4, tag="qT", name=f"qT{t}")
            nc.scalar.activation(qT[:, :, :], tT[:, :, :], ACTF.Copy, bias=-MAGIC)
            s["qT"] = qT

        def phase_d(t):
            # M2 fp8 DoubleRow
            s = S[t]
            qT = s["qT"]
            g = ps_g.tile([P, 2, 512], dt.float32, tag="g", name=f"g{t}")
            for kk in range(NCHUNK // 2):
                for oh in range(2):
                    nc.tensor.matmul(
                        g[:, oh, :], qT[:, 2 * kk : 2 * kk + 2, :],
                        ternT[:, 2 * kk : 2 * kk + 2, oh * 512 : (oh + 1) * 512],
                        start=(kk == 0), stop=(kk == NCHUNK // 2 - 1),
                        perf_mode=mybir.MatmulPerfMode.DoubleRow,
                        skip_group_check=True,
                    )
            s["g"] = g

        def phase_e(t):
            # epilogue + store
            s = S[t]
            g, sc = s["g"], s["sc"]
            o_t = opool.tile([P, NCHUNK * P], dt.float32, tag="o", name=f"o{t}")
            for oh in range(2):
                nc.vector.scalar_tensor_tensor(
                    o_t[:, oh * 512 : (oh + 1) * 512], g[:, oh, :], sc[:],
                    wsb[:, oh * 512 : (oh + 1) * 512], ALU.mult, ALU.mult,
                )
            nc.sync.dma_start(out[t * P : (t + 1) * P, :], o_t[:])
            S[t].clear()

        # Software pipeline: each round touches 5 tiles at different stages
        # so every engine's in-order stream has its dependencies satisfied
        # a round in advance (epilogue first frees g for this round's M2).
        phase_load(0)
        phase_load(1)
        for i in range(ST + 4):
            if i >= 4:
                phase_e(i - 4)
            if i + 2 < ST:
                phase_load(i + 2)
            if i < ST:
                phase_a(i)
            if 1 <= i < ST + 1:
                phase_b(i - 1)
            if 2 <= i < ST + 2:
                phase_c(i - 2)
            if 3 <= i < ST + 3:
                phase_d(i - 3)

    nc.finalize()
    return nc


def _get_nc():
    if "nc" not in _CACHE:
        _CACHE["nc"] = _build()
    return _CACHE["nc"]


def _make_hmx() -> np.ndarray:
    hm = (_sylvester(7).astype(np.float32) / np.float32(32.0)).astype(np.float32)
    return np.ascontiguousarray(
        np.stack([hm, hm, hm, -hm], axis=1)
    )  # [128, 4, 128]


def _in_maps(x: np.ndarray, weight: np.ndarray) -> list:
    hmx = _make_hmx()
    w32 = np.ascontiguousarray(weight, dtype=np.float32)
    return [
        {"x": np.ascontiguousarray(x[i]), "w": w32, "hmx": hmx} for i in range(8)
    ]


def kernel(x: np.ndarray, weight: np.ndarray) -> np.ndarray:
    from concourse.bass_utils import run_bass_kernel_spmd

    assert x.shape == (8, ST * P, NCHUNK * P) and x.dtype == np.float32
    assert weight.shape == (NCHUNK * P, NCHUNK * P)

    nc = _get_nc()
    res = run_bass_kernel_spmd(nc, _in_maps(x, weight), core_ids=list(range(8)))
    return np.stack([res.results[i]["out"] for i in range(8)], axis=0)
